# revision 9
# baseline (speedup 1.0000x reference)
"""Spectral pooling (FFT2 -> crop low freqs -> IFFT2) as dense DFT matmuls on TRN2.

Input  x: (32, 256, 64, 64) fp32  -- channels 0:128 real part, 128:256 imag part
Output y: (32, 256, 32, 32) fp32

Math: per complex image X (64x64), Y = A @ X @ A.T with
  A = sqrt(1/(64*32)) * IDFT32 @ Crop @ DFT64   (32x64 complex)
Sharding: batch dim across 8 cores (4 batches/core), no communication.

HBM I/O is bf16 and pre-packed on the host into the exact SBUF layouts so
every DMA moves >=4KB contiguous runs per partition at full bus rate:
  x_dev[b]  [128, 8192] = x[b] with partitions (xc, h), cols (c, w)
  y_dev[b]  [128, 2048] = raw stage-2 results; host unscrambles + upcasts.
(The fp32->bf16 cast is numerically identical to the in-DMA cast the previous
revision used; matmuls consumed bf16 either way.)

Per complex-channel pair (jj half of a quad q) stage 1 computes, in ONE
matmul, P = A X for two images j=0,1 (complex combine happens inside the
K contraction -- partitions hold (xc, h), the moving R1 = [[ArT,AiT],
[-AiT,ArT]] streams only 64 cols):
  psum1[(j,w), (pc,p)] = P^j_pc[p,w]
Stage 2 accumulates 2 matmuls (Pr-slice x D2r + Pi-slice x D2i, block-diag
over j) -> psum2[(jj,p), (j,yc,p2)] = Y. 96 streamed cols/image vs 128 for
the 2-matmul-per-stage scheme.

PSUM->SBUF copies are batched into full 2KB banks ([128,512]) and split
across DVE and ACT so neither exceeds the DMA bottleneck (~29us/rep).
"""

import math

import numpy as np

from concourse import bass, mybir
from concourse.bass_utils import run_bass_kernel_spmd
from concourse.tile import TileContext

N_CORES = 8
B_FULL, C2, H, W = 32, 256, 64, 64
HP, WP = 32, 32
BPC = B_FULL // N_CORES  # batches per core

F32 = mybir.dt.float32
BF16 = mybir.dt.bfloat16


def _split_multi_waits(nc):
    """This walrus build rejects instructions carrying more than one semaphore
    wait. Hoist extra waits onto same-engine NOPs inserted just before the
    instruction (engine queues execute in order, so blocking is equivalent)."""
    n_split = 0
    for f in nc.m.functions:
        for bb in f.blocks:
            insts = bb.instructions
            out = []
            for inst in insts:
                si = inst.sync_info
                waits = list(si.on_wait) if si and si.on_wait else []
                if len(waits) > 1:
                    si.on_wait = waits[-1:]
                    for w in waits[:-1]:
                        nop = mybir.InstNoOp(
                            name=nc.get_next_instruction_name(),
                            ins=[],
                            outs=[],
                            engine=inst.engine,
                            sync_info=mybir.SyncInfo(on_wait=[w], on_update=[]),
                        )
                        out.append(nop)
                        n_split += 1
                out.append(inst)
            if len(out) != len(insts):
                insts[:] = out
    return n_split


def _a_matrix():
    topf = int(math.ceil(H * 0.5 / 2))  # 16
    midf = H // 2 + topf  # 48
    F = np.exp(-2j * np.pi * np.outer(np.arange(H), np.arange(H)) / H)
    G = np.exp(2j * np.pi * np.outer(np.arange(HP), np.arange(HP)) / HP)
    keep = list(range(topf)) + list(range(midf, H))
    S = np.zeros((HP, H))
    S[np.arange(HP), keep] = 1
    return (G @ S @ F) / np.sqrt(H * W * HP * WP) ** 0.5


def _dft_constants():
    """[128, 320] f32: R1 (64 cols) | D2r (128) | D2i (128)."""
    A = _a_matrix()
    ArT = A.real.astype(np.float32).T  # [64, 32]
    AiT = A.imag.astype(np.float32).T

    R1 = np.block([[ArT, AiT], [-AiT, ArT]])  # [128(xc,h), 64(pc,p)]
    C2r = np.concatenate([ArT, AiT], axis=1)  # [64(w), 64(yc,p2)]
    C2i = np.concatenate([-AiT, ArT], axis=1)
    D2r = np.zeros((128, 128), np.float32)
    D2i = np.zeros((128, 128), np.float32)
    D2r[:64, :64] = C2r
    D2r[64:, 64:] = C2r
    D2i[:64, :64] = C2i
    D2i[64:, 64:] = C2i
    return np.concatenate([R1, D2r, D2i], axis=1)


def build_program(reps: int = 1, split_waits: bool = True):
    """reps > 1 unrolls the whole pipeline in-NEFF over the same data so the
    marginal cost per rep can be measured without the ~65ms axon dispatch
    overhead."""
    nc = bass.Bass("TRN2", target_bir_lowering=False, debug=False)
    x = nc.dram_tensor("x", [BPC, 128, 8192], BF16, kind="ExternalInput").ap()
    dm = nc.dram_tensor("dmats", [128, 320], BF16, kind="ExternalInput").ap()
    y = nc.dram_tensor("y", [BPC, 128, 2048], BF16, kind="ExternalOutput").ap()

    with TileContext(nc) as tc:
        with (
            tc.tile_pool(name="consts", bufs=1) as cpool,
            tc.tile_pool(name="inp", bufs=2) as ipool,
            tc.tile_pool(name="sb1", bufs=3) as s1pool,
            tc.tile_pool(name="sbout", bufs=2) as opool,
            tc.tile_pool(name="ps1", bufs=3, space="PSUM") as p1pool,
            tc.tile_pool(name="ps2", bufs=2, space="PSUM") as p2pool,
        ):
            dmb = cpool.tile([128, 320], BF16, tag="dmb")
            nc.sync.dma_start(out=dmb, in_=dm)
            r1 = dmb[:, 0:64]
            d2r = dmb[:, 64:192]
            d2i = dmb[:, 192:320]

            for b in [b for _ in range(reps) for b in range(BPC)]:
                tin = ipool.tile([128, 8192], BF16, tag="tin")
                nc.gpsimd.dma_start(out=tin, in_=x[b])
                sb_out = opool.tile([128, 2048], BF16, tag="sb_out")
                ps2 = None
                for sg in range(8):  # stage-1 groups of 4 quads
                    ps1 = p1pool.tile([128, 512], F32, tag="ps1")
                    for q4 in range(4):
                        q = 4 * sg + q4
                        for jj in range(2):
                            lo = 256 * q + 128 * jj
                            nc.tensor.matmul(
                                out=ps1[:, 128 * q4 + 64 * jj :
                                        128 * q4 + 64 * jj + 64],
                                lhsT=tin[:, lo : lo + 128],
                                rhs=r1,
                                start=True,
                                stop=True,
                                tile_position=(0, 0),
                            )
                    # copy out deinterleaving pc: ps1 cols (q,jj,pc,pp) ->
                    # sb1 cols (q,pc,jj,pp), one 3D-AP copy per pc so the
                    # stage-2 lhsT slices below are contiguous (the PE and
                    # ACT ISAs reject >1-free-dim / >3D APs respectively).
                    sb1 = s1pool.tile([128, 512], BF16, tag="sb1")
                    ps1v = ps1.rearrange(
                        "z (q jj pc pp) -> z q jj pc pp", q=4, jj=2, pc=2, pp=32
                    )
                    sb1v = sb1.rearrange(
                        "z (q pc jj pp) -> z q pc jj pp", q=4, pc=2, jj=2, pp=32
                    )
                    eng = nc.vector if sg % 2 == 0 else nc.scalar
                    for pc in range(2):
                        if sg % 2 == 0:
                            eng.tensor_copy(
                                out=sb1v[:, :, pc], in_=ps1v[:, :, :, pc]
                            )
                        else:
                            eng.copy(out=sb1v[:, :, pc], in_=ps1v[:, :, :, pc])
                    if sg % 2 == 0:
                        ps2 = p2pool.tile([128, 512], F32, tag="ps2")
                    for q4 in range(4):
                        qpar = q4 % 2
                        slot = 2 * (sg % 2) + q4 // 2
                        out_ap = ps2[64 * qpar : 64 * qpar + 64,
                                     128 * slot : 128 * slot + 128]
                        nc.tensor.matmul(
                            out=out_ap,
                            lhsT=sb1[:, 128 * q4 : 128 * q4 + 64],
                            rhs=d2r,
                            start=True,
                            stop=False,
                            tile_position=(0, 64 * qpar),
                        )
                        nc.tensor.matmul(
                            out=out_ap,
                            lhsT=sb1[:, 128 * q4 + 64 : 128 * q4 + 128],
                            rhs=d2i,
                            start=False,
                            stop=True,
                            tile_position=(0, 64 * qpar),
                        )
                    if sg % 2 == 1:
                        sgp = sg // 2
                        o = sb_out[:, 512 * sgp : 512 * sgp + 512]
                        if sgp % 2 == 0:
                            nc.scalar.copy(out=o, in_=ps2)
                        else:
                            nc.vector.tensor_copy(out=o, in_=ps2)
                nc.sync.dma_start(out=y[b], in_=sb_out)
    if split_waits:
        _split_multi_waits(nc)
    return nc


def _bf16(a: np.ndarray) -> np.ndarray:
    return a.astype(mybir.dt.np(BF16))


def _pack_x(x_shard: np.ndarray) -> np.ndarray:
    """[BPC, 256, 64, 64] f32 -> [BPC, 128, 8192] bf16, partitions (xc, h),
    cols (c, w)."""
    b = x_shard.shape[0]
    xr = x_shard.reshape(b, 2, 128, 64, 64).transpose(0, 1, 3, 2, 4)
    return _bf16(np.ascontiguousarray(xr).reshape(b, 128, 8192))


def _unpack_y(y_dev: np.ndarray) -> np.ndarray:
    """[BPC, 128, 2048] bf16 -> [BPC, 256, 32, 32] f32.

    part = 64*qpar + 32*jj + p ; col = 512*sgp + 128*(2*sh+sl) + 64*j
    + 32*yc + p2 ; channel = 128*yc + 32*sgp + 16*sh + 8*sl + 4*qpar
    + 2*jj + j."""
    b = y_dev.shape[0]
    a = y_dev.astype(np.float32).reshape(b, 2, 2, 32, 4, 2, 2, 2, 2, 32)
    #                                       qpar jj p sgp sh sl j yc p2
    a = a.transpose(0, 8, 4, 5, 6, 1, 2, 7, 3, 9)  # b yc sgp sh sl qpar jj j p p2
    return np.ascontiguousarray(a).reshape(b, 256, 32, 32)


def _make_in_map(x_shard: np.ndarray, dmats: np.ndarray) -> dict:
    return {"x": _pack_x(x_shard), "dmats": _bf16(dmats)}


_CACHED = {}


def _get_program():
    if "nc" not in _CACHED:
        _CACHED["nc"] = build_program()
        _CACHED["consts"] = _dft_constants()
    return _CACHED["nc"], _CACHED["consts"]


def kernel(x: np.ndarray) -> np.ndarray:
    assert x.shape == (B_FULL, C2, H, W) and x.dtype == np.float32
    nc, dmats = _get_program()
    in_maps = [
        _make_in_map(x[BPC * k : BPC * (k + 1)], dmats)
        for k in range(N_CORES)
    ]
    res = run_bass_kernel_spmd(nc, in_maps, list(range(N_CORES)))
    out = np.concatenate(
        [_unpack_y(np.asarray(res.results[k]["y"])) for k in range(N_CORES)],
        axis=0,
    )
    return out.astype(np.float32, copy=False)


if __name__ == "__main__":
    rng = np.random.default_rng(0)
    x = rng.standard_normal((B_FULL, C2, H, W)).astype(np.float32)
    y = kernel(x)
    print("kernel output", y.shape, y.dtype)
